# revision 46
# baseline (speedup 1.0000x reference)
"""Trainium2 Bass kernel for nn_Block_36575941492917 (ViG / gnn_message_passing).

Data-parallel over batch: 16 images -> 8 cores x 2 images.

Per-image pipeline (activations c-major (C, N) in SBUF):
  1. conv1x1 C->C + folded BN          (PE matmul + ACT bias copyback)
  2. 2x2 avg pool -> Y4 (= 4*Y)        (DVE strided adds)
  3. transpose Y4 tiles, column norms  (PE transpose + ACT square/accum)
     -> Yn2 = 2*Y/||Y|| c-major, yt = Y bf16 row-major table in DRAM
  4. TAB6: d=6 packed bf16 gather table (128 partitions = 8 groups of 16,
     groups 0-3 img0 replicas, 4-7 img1; partition pp holds ch e*16+pp)
  5. per 112-node tile, nodes processed in sigma-permuted partition order
     sigma(p) = (p%7)*16 + p//7 (rel pre-permuted on host):
     scores s = 2<xn,yn> - 1 - rel (PE matmul, ACT scale, DVE subtract)
  6. top-9 via DVE max8/max_index/match_replace -> u16 (112,9) contiguous
     DRAM write; thanks to sigma this IS the 16-wrapped index layout
  7. per group-of-8-tiles: 2 contiguous index loads, ONE gpsimd.ap_gather
     (d=6, 1008 idxs) = 9 neighbors x 96 ch x 8 tiles; DVE tree-max over
     k on contiguous 96-elem slices; PE mini-transposes -> c-major;
     msg = max_k(y_j) - hx into persistent SBUF Msg
  8. gc conv (2C->2C) + BN+GELU; fc2 + BN, residual -> score_map
  9. FFN (C->4C GELU 4C->C, BNs folded) + residual -> out
"""

import numpy as np

import concourse.bass as bass
import concourse.tile as tile
from concourse import bacc, mybir
from concourse.bass_utils import run_bass_kernel_spmd
from concourse.masks import make_identity

F32 = mybir.dt.float32
BF16 = mybir.dt.bfloat16
U32 = mybir.dt.uint32
U16 = mybir.dt.uint16
I16 = mybir.dt.int16
AF = mybir.ActivationFunctionType
OP = mybir.AluOpType
AX = mybir.AxisListType

B, C, H, W = 16, 96, 56, 56
N = H * W            # 3136
NR = N // 4          # 784
KNN = 9
NCORES = 8
IPC = B // NCORES    # 2 images per core
NT = 112             # n-tile rows for the knn/topk phase
NTILES = N // NT     # 28
CHK = 448            # n-chunk for conv phases
NCHK = N // CHK      # 7
C2 = 2 * C           # 192
C4 = 4 * C           # 384
EPS = 1e-5
NEG = -1.0e30
D6 = 6               # channels packed per gathered element
PPT = C // D6        # 16 partitions per image-tile in the gather call
TPC = 4              # tiles per call per image
CALLS = NTILES // TPC  # 7 gather calls per body
NK = NT * KNN        # 1008 indices per tile


def _build_nc(reps: int = 1, ndev: int = NCORES):
    nc = bacc.Bacc("TRN2", target_bir_lowering=False, debug=False,
                   num_devices=ndev)

    # ---- DRAM I/O ----
    xs = nc.dram_tensor("xs", [IPC, C, N], F32, kind="ExternalInput")
    relp = nc.dram_tensor("relp", [NTILES, NT, NR], F32, kind="ExternalInput")
    w1t = nc.dram_tensor("w1t", [C, C], F32, kind="ExternalInput")
    b1 = nc.dram_tensor("b1", [C, 1], F32, kind="ExternalInput")
    w2ta = nc.dram_tensor("w2ta", [C, C2], BF16, kind="ExternalInput")
    w2tb = nc.dram_tensor("w2tb", [C, C2], BF16, kind="ExternalInput")
    b2 = nc.dram_tensor("b2", [C, 2], F32, kind="ExternalInput")
    w3t = nc.dram_tensor("w3t", [C, 2 * C], BF16, kind="ExternalInput")
    b3 = nc.dram_tensor("b3", [C, 1], F32, kind="ExternalInput")
    w4t = nc.dram_tensor("w4t", [C, C4], BF16, kind="ExternalInput")
    b4 = nc.dram_tensor("b4", [128, 3], F32, kind="ExternalInput")
    w5t = nc.dram_tensor("w5t", [128, 3 * C], BF16, kind="ExternalInput")
    b5 = nc.dram_tensor("b5", [C, 1], F32, kind="ExternalInput")
    out_d = nc.dram_tensor("out", [IPC, C, N], F32, kind="ExternalOutput")
    # idxw[i, nt] flat (p*9+k) == wrapped layout (pw*63 + jj*9 + k)
    idxw = nc.dram_tensor("idxw", [IPC, NTILES, NT * KNN], U16)

    with tile.TileContext(nc) as tc:
        _emit(nc, tc, reps, xs, relp, w1t, b1, w2ta, w2tb, b2, w3t, b3,
              w4t, b4, w5t, b5, out_d, idxw)
    nc.compile()
    return nc


def _emit(nc, tc, reps, xs, relp, w1t, b1, w2ta, w2tb, b2, w3t, b3,
          w4t, b4, w5t, b5, out_d, idxw):
    from contextlib import ExitStack
    ctx = ExitStack()
    with ctx:
        singles = ctx.enter_context(tc.tile_pool(name="singles", bufs=1))
        resid = ctx.enter_context(tc.tile_pool(name="resid", bufs=1))

        # identities for PE transposes
        id_f32 = singles.tile([128, 128], F32, tag="id_f32")
        make_identity(nc, id_f32)
        id_bf16 = singles.tile([128, 128], BF16, tag="id_bf16")
        make_identity(nc, id_bf16)
        # partition-selection matrices for the TAB6 build:
        # Psel[c, e, p] = 1 iff c == e*16 + p%16
        Psel = singles.tile([C, D6, 128], BF16, tag="Psel")
        for e in range(D6):
            for g in range(8):
                nc.vector.tensor_copy(
                    Psel[:, e, g * PPT:(g + 1) * PPT],
                    id_bf16[0:C, e * PPT:(e + 1) * PPT])

        # weights -> SBUF
        def load(name, dram, shape, dt=F32):
            t = singles.tile(shape, dt, tag=name)
            nc.sync.dma_start(out=t[:], in_=dram[:])
            return t

        w1t_s = load("w1t", w1t, [C, C])
        b1_s = load("b1", b1, [C, 1])
        w2ta_s = load("w2ta", w2ta, [C, C2], BF16)
        w2tb_s = load("w2tb", w2tb, [C, C2], BF16)
        b2_s = load("b2", b2, [C, 2])
        w3t_s = load("w3t", w3t, [C, 2 * C], BF16)
        b3_s = load("b3", b3, [C, 1])
        w4t_s = load("w4t", w4t, [C, C4], BF16)
        b4_s = load("b4", b4, [128, 3])
        w5t_s = load("w5t", w5t, [128, 3 * C], BF16)
        b5_s = load("b5", b5, [C, 1])

        # persistent per-image activations (c-major)
        X = [resid.tile([C, N], F32, tag=f"X{i}", name=f"X{i}")
             for i in range(IPC)]
        Hx = [resid.tile([C, N], F32, tag=f"Hx{i}", name=f"Hx{i}")
              for i in range(IPC)]
        Msg = [resid.tile([C, N], BF16, tag=f"Msg{i}", name=f"Msg{i}")
               for i in range(IPC)]
        Hxb = [resid.tile([C, N], BF16, tag=f"Hxb{i}", name=f"Hxb{i}")
               for i in range(IPC)]
        Smapb = [resid.tile([C, N], BF16, tag=f"Sb{i}", name=f"Sb{i}")
                 for i in range(IPC)]
        Smap = [resid.tile([C, N], F32, tag=f"S{i}", name=f"S{i}")
                for i in range(IPC)]
        Yn2 = [resid.tile([C, NR], F32, tag=f"Yn2{i}", name=f"Yn2{i}")
               for i in range(IPC)]
        TAB6 = resid.tile([128, NR, D6], BF16, tag="TAB6", name="TAB6")

        def body(_iv=None):
            # ---------------- phase A: conv1, pool, normalize, tables -----
            with (
                tc.tile_pool(name="ptmp", bufs=2) as ptmp,
                tc.tile_pool(name="psA", bufs=2, space="PSUM") as psA,
                tc.tile_pool(name="psB", bufs=2, space="PSUM") as psB,
                tc.tile_pool(name="psC", bufs=1, space="PSUM") as psC,
            ):
                for i in range(IPC):
                    nc.sync.dma_start(out=X[i][:], in_=xs[i, :, :])
                    # conv1 + BN fold
                    for ch in range(NCHK):
                        sl = bass.ts(ch, CHK)
                        ps = psA.tile([C, CHK], F32, tag="conv1")
                        nc.tensor.matmul(ps[:], lhsT=w1t_s[:], rhs=X[i][:, sl],
                                         start=True, stop=True)
                        nc.scalar.activation(Hx[i][:, sl], ps[:], AF.Identity,
                                             bias=b1_s[:, 0:1], scale=1.0)
                        nc.scalar.activation(Hxb[i][:, sl], ps[:], AF.Identity,
                                             bias=b1_s[:, 0:1], scale=1.0)
                    # 2x2 avg pool (x4)
                    t1 = ptmp.tile([C, N // 2], F32, tag="t1")
                    hv = Hx[i].rearrange("p (x two) -> p x two", two=2)
                    nc.vector.tensor_tensor(t1[:], hv[:, :, 0], hv[:, :, 1],
                                            op=OP.add)
                    y4 = ptmp.tile([C, NR], F32, tag="y4")
                    tv = t1.rearrange("p (h two w) -> p h two w", two=2, w=28)
                    nc.vector.tensor_tensor(y4[:], tv[:, :, 0, :], tv[:, :, 1, :],
                                            op=OP.add)
                    # per-m-column norms + yn2
                    for mt in range(NR // NT):  # 7 tiles of 112
                        msl = bass.ts(mt, NT)
                        pt = psA.tile([NT, C], F32, tag="ytr")
                        nc.tensor.transpose(pt[:], y4[:, msl], id_f32[:C, :C])
                        sq = ptmp.tile([NT, C], F32, tag="sq")
                        ssq = ptmp.tile([NT, 1], F32, tag="ssq")
                        nc.scalar.activation(sq[:], pt[:], AF.Square,
                                             accum_out=ssq[:])
                        rt = ptmp.tile([NT, 1], F32, tag="rt")
                        nc.scalar.activation(rt[:], ssq[:], AF.Sqrt, scale=0.25)
                        rec = ptmp.tile([NT, 1], F32, tag="rec")
                        nc.vector.reciprocal(rec[:], rt[:])
                        ynt = ptmp.tile([NT, C], F32, tag="ynt")
                        nc.scalar.activation(ynt[:], pt[:], AF.Copy, bias=0.0,
                                             scale=rec[:])
                        pb = psB.tile([C, NT], F32, tag="ynb")
                        nc.tensor.transpose(pb[:], ynt[:], id_f32[:NT, :NT])
                        nc.scalar.activation(Yn2[i][:, msl], pb[:], AF.Copy,
                                             bias=0.0, scale=1.0)
                    # TAB6 build on PE: partition pp of each 16-group holds
                    # channels e*16+pp of Y (= y4 * 0.25), bf16
                    y4b = ptmp.tile([C, NR], BF16, tag="y4b")
                    nc.scalar.activation(y4b[:], y4[:], AF.Copy, bias=0.0,
                                         scale=0.25)
                    for e in range(D6):
                        pt6 = psC.tile([128, NR], F32, tag="pt6")
                        hsl = slice(i * 64, (i + 1) * 64)
                        nc.tensor.matmul(
                            pt6[hsl, 0:512], lhsT=Psel[:, e, hsl],
                            rhs=y4b[:, 0:512], start=True, stop=True,
                            tile_position=(0, i * 64))
                        nc.tensor.matmul(
                            pt6[hsl, 512:NR], lhsT=Psel[:, e, hsl],
                            rhs=y4b[:, 512:NR], start=True, stop=True,
                            tile_position=(0, i * 64))
                        nc.vector.tensor_copy(
                            TAB6[hsl, :, e], pt6[hsl, :])

            # -------- phase E+F interleaved: per gather-call j, the conv
            # chunk j of both images runs on PE/ACT while call j+1's
            # topk/gather occupies DVE/Pool
            with (
                tc.tile_pool(name="relp_p", bufs=2) as relpool,
                tc.tile_pool(name="sp", bufs=2) as sp,
                tc.tile_pool(name="ip", bufs=3) as ip,
                tc.tile_pool(name="gp", bufs=2) as gp,
                tc.tile_pool(name="wp", bufs=2) as wp,
                tc.tile_pool(name="mp", bufs=2) as mp,
                tc.tile_pool(name="ctmp", bufs=2) as ctmp,
                tc.tile_pool(name="psS", bufs=1, space="PSUM") as psS,
                tc.tile_pool(name="psT", bufs=2, space="PSUM") as psT,
                tc.tile_pool(name="psM", bufs=1, space="PSUM") as psM,
                tc.tile_pool(name="psF4", bufs=2, space="PSUM") as psF4,
            ):
                def emitF(i, ch):
                    sl = bass.ts(ch, CHK)
                    # gc conv: out 192 ch in two groups of 96
                    g1 = ctmp.tile([C, 2, CHK], BF16, tag="g1")
                    for gi in range(2):
                        gsl = bass.ts(gi, C)
                        pg = psF4.tile([128, CHK], F32, tag="pf4")
                        nc.tensor.matmul(pg[:C, :], lhsT=w2ta_s[:, gsl],
                                         rhs=Hxb[i][:, sl],
                                         start=True, stop=False)
                        nc.tensor.matmul(pg[:C, :], lhsT=w2tb_s[:, gsl],
                                         rhs=Msg[i][:, sl],
                                         start=False, stop=True)
                        nc.scalar.activation(g1[:, gi, :], pg[:C, :], AF.Gelu,
                                             bias=b2_s[:, gi:gi + 1])
                    # fc2 + residual -> score map
                    pf = psF4.tile([128, CHK], F32, tag="pf4")
                    nc.tensor.matmul(pf[:C, :], lhsT=w3t_s[:, 0:C],
                                     rhs=g1[:, 0, :], start=True, stop=False)
                    nc.tensor.matmul(pf[:C, :], lhsT=w3t_s[:, C:2 * C],
                                     rhs=g1[:, 1, :], start=False, stop=True)
                    t3 = ctmp.tile([C, CHK], F32, tag="t3")
                    nc.scalar.activation(t3[:], pf[:C, :], AF.Identity,
                                         bias=b3_s[:, 0:1])
                    nc.vector.tensor_tensor(Smap[i][:, sl], t3[:],
                                            X[i][:, sl], op=OP.add)
                    nc.scalar.activation(Smapb[i][:, sl], Smap[i][:, sl],
                                         AF.Copy, bias=0.0, scale=1.0)
                    # FFN
                    u = ctmp.tile([128, 3, CHK], BF16, tag="u")
                    for gi in range(3):
                        pu = psF4.tile([128, CHK], F32, tag="pf4")
                        nc.tensor.matmul(pu[:], lhsT=w4t_s[:, bass.ts(gi, 128)],
                                         rhs=Smapb[i][:, sl],
                                         start=True, stop=True)
                        nc.scalar.activation(u[:, gi, :], pu[:], AF.Gelu,
                                             bias=b4_s[:, gi:gi + 1])
                    pv = psF4.tile([128, CHK], F32, tag="pf4")
                    for gi in range(3):
                        nc.tensor.matmul(pv[:C, :], lhsT=w5t_s[:, bass.ts(gi, C)],
                                         rhs=u[:, gi, :],
                                         start=(gi == 0), stop=(gi == 2))
                    t5 = ctmp.tile([C, CHK], F32, tag="t5")
                    nc.scalar.activation(t5[:], pv[:C, :], AF.Identity,
                                         bias=b5_s[:, 0:1])
                    ot = ctmp.tile([C, CHK], F32, tag="ot")
                    nc.vector.tensor_tensor(ot[:], t5[:], Smap[i][:, sl],
                                            op=OP.add)
                    nc.sync.dma_start(out=out_d[i, :, sl], in_=ot[:])

                for j in range(CALLS):
                    for tg in range(TPC):
                        nt = j * TPC + tg
                        nsl = bass.ts(nt, NT)
                        rel_t = relpool.tile([NT, NR], F32, tag="rel")
                        nc.sync.dma_start(out=rel_t[:], in_=relp[nt, :, :])
                        for i in range(IPC):
                            # sigma-permuted columns: part p <- node
                            # sigma(p) = (p%7)*16 + p//7, materialized
                            hxv = Hx[i][:, nsl].rearrange(
                                "c (b a) -> c a b", b=CALLS)
                            hxp = ip.tile([C, NT], F32, tag="hxp")
                            hpv = hxp.rearrange("c (a b) -> c a b", a=PPT)
                            nc.scalar.activation(hpv[:], hxv, AF.Copy,
                                                 bias=0.0, scale=1.0)
                            hxp = hxp[:]
                            # 1/||hx col||
                            pht = psT.tile([NT, C], F32, tag="ht")
                            nc.tensor.transpose(pht[:], hxp, id_f32[:C, :C])
                            hsq = ip.tile([NT, C], F32, tag="hsq")
                            hssq = ip.tile([NT, 1], F32, tag="hssq")
                            nc.scalar.activation(hsq[:], pht[:], AF.Square,
                                                 accum_out=hssq[:])
                            hrt = ip.tile([NT, 1], F32, tag="hrt")
                            nc.scalar.activation(hrt[:], hssq[:], AF.Sqrt)
                            invr = ip.tile([NT, 1], F32, tag="invr")
                            nc.vector.reciprocal(invr[:], hrt[:])
                            # scores (constant -1 shift dropped: only the
                            # ordering matters for top-k)
                            ps = psS.tile([NT, NR], F32, tag="s")
                            nc.tensor.matmul(ps[:, 0:512], lhsT=hxp,
                                             rhs=Yn2[i][:, 0:512],
                                             start=True, stop=True)
                            nc.tensor.matmul(ps[:, 512:NR], lhsT=hxp,
                                             rhs=Yn2[i][:, 512:NR],
                                             start=True, stop=True)
                            s = sp.tile([NT, NR], F32, tag="s")
                            nc.vector.scalar_tensor_tensor(
                                s[:], ps[:], invr[:], rel_t[:],
                                op0=OP.mult, op1=OP.subtract)
                            # top-9 -> ifull cols 0:9 u16
                            ifull = ip.tile([NT, 16], U16, tag="ifull")
                            m8 = ip.tile([NT, 8], F32, tag="m8")
                            nc.vector.max(m8[:], s[:])
                            nc.vector.max_index(ifull[:, 0:8], m8[:], s[:])
                            srep = sp.tile([NT, NR], F32, tag="srep")
                            nc.vector.match_replace(srep[:], in_to_replace=m8[:],
                                                    in_values=s[:],
                                                    imm_value=NEG)
                            m8b = ip.tile([NT, 8], F32, tag="m8b")
                            nc.vector.max(m8b[:], srep[:])
                            nc.vector.max_index(
                                ifull[:, 8:16],
                                m8b[:, 0:1].to_broadcast([NT, 8]), s[:])
                            nc.sync.dma_start(
                                out=idxw[i, nt, :].rearrange(
                                    "(p k) -> p k", k=KNN),
                                in_=ifull[:, 0:KNN])
                    # wrapped index load: one contiguous DMA per image
                    w = wp.tile([128, 63], U16, tag="w")
                    for i in range(IPC):
                        src = idxw[i, j * TPC:(j + 1) * TPC, :].rearrange(
                            "tg (pw c) -> (tg pw) c", c=63)
                        nc.sync.dma_start(out=w[i * 64:(i + 1) * 64, :],
                                          in_=src)
                    # ONE gather for 8 image-tiles
                    g6 = gp.tile([128, NK, D6], BF16, tag="g6")
                    nc.gpsimd.ap_gather(
                        g6[:], TAB6[:], w[:].bitcast(I16),
                        channels=128, num_elems=NR, d=D6, num_idxs=NK)
                    # tree-max over k on contiguous 96-elem slices
                    # g6 free = (jj:864, k:96, (pw e):1)
                    gk = g6.rearrange("p (jj k m) e -> p jj k (m e)",
                                      jj=CALLS, k=KNN)
                    acc1 = mp.tile([128, NT * D6], F32, tag="acc1")
                    a1 = acc1.rearrange("p (jj m) -> p jj m", jj=CALLS)
                    nc.vector.tensor_tensor(a1[:], gk[:, :, 0, :],
                                            gk[:, :, 1, :], op=OP.max)
                    for kk in range(2, 8):
                        nc.vector.tensor_tensor(a1[:], a1[:], gk[:, :, kk, :],
                                                op=OP.max)
                    mx = mp.tile([128, NT * D6], BF16, tag="mx")
                    mxv = mx.rearrange("p (jj m) -> p jj m", jj=CALLS)
                    nc.vector.tensor_tensor(mxv[:], a1[:], gk[:, :, 8, :],
                                            op=OP.max)
                    # per 32-partition pair (2 image-tiles): 6 transposes
                    # into one PSUM tile, then per tile ACT+transpose -> Msg
                    mxv = mx.rearrange("p (n e) -> p n e", e=D6)
                    for q in range(TPC):
                        q0 = 32 * q
                        pq = psM.tile([NT, D6 * 32], BF16, tag="pq")
                        for e in range(D6):
                            nc.tensor.transpose(
                                pq[:, e * 32:(e + 1) * 32],
                                mxv[q0:q0 + 32, :, e],
                                id_bf16[q0:q0 + 32, q0:q0 + 32],
                                tile_position=(q0, 0))
                        pqv = pq.rearrange("p (e w) -> p e w", w=32)
                        for h in range(2):
                            g = 2 * q + h
                            i, tg = g // TPC, g % TPC
                            nt = j * TPC + tg
                            nsl = bass.ts(nt, NT)
                            msgT = mp.tile([NT, C], BF16, tag="msgT")
                            mtv = msgT.rearrange("p (e pp) -> p e pp", e=D6)
                            nc.scalar.activation(
                                mtv[:], pqv[:, :, 16 * h:16 * h + PPT],
                                AF.Copy, bias=0.0, scale=1.0)
                            pmt2 = psM.tile([C, NT], BF16, tag="pmt2")
                            nc.tensor.transpose(pmt2[:], msgT[:],
                                                id_bf16[:NT, :NT])
                            nc.vector.tensor_tensor(Msg[i][:, nsl], pmt2[:],
                                                    Hx[i][:, nsl],
                                                    op=OP.subtract)
                    # conv/FFN chunk j of both images
                    for i in range(IPC):
                        emitF(i, j)

        if reps == 1:
            body()
        else:
            with tc.For_i(0, reps, 1) as iv:
                body(iv)


# ------------------------- host side ---------------------------------------

def _fold_bn(g, b, m, v):
    inv = g / np.sqrt(v + EPS)
    return inv, b - m * inv


def _prep_weights(inp):
    f32 = np.float32
    o = {}
    inv1, sh1 = _fold_bn(inp["g_bn1_g"], inp["g_bn1_b"], inp["g_bn1_m"],
                         inp["g_bn1_v"])
    w1 = inp["g_fc1_w"] * inv1[:, None]
    b1 = inp["g_fc1_b"] * inv1 + sh1
    o["w1t"] = np.ascontiguousarray(w1.T, f32)
    o["b1"] = np.ascontiguousarray(b1[:, None], f32)

    inv2, sh2 = _fold_bn(inp["gc_bn_g"], inp["gc_bn_b"], inp["gc_bn_m"],
                         inp["gc_bn_v"])
    w2 = inp["gc_w"] * inv2[:, None]
    b2v = inp["gc_b"] * inv2 + sh2
    perm = np.concatenate([np.arange(0, C2, 2), np.arange(1, C2, 2)])
    w2p = w2[:, perm]          # stacked [hx; msg] input order
    w2T = w2p.T                # (192 in, 192 out)
    import ml_dtypes
    bf16 = ml_dtypes.bfloat16
    o["w2ta"] = np.ascontiguousarray(w2T[:C, :]).astype(bf16)
    o["w2tb"] = np.ascontiguousarray(w2T[C:, :]).astype(bf16)
    o["b2"] = np.ascontiguousarray(
        np.stack([b2v[:C], b2v[C:]], axis=1), f32)

    inv3, sh3 = _fold_bn(inp["g_bn2_g"], inp["g_bn2_b"], inp["g_bn2_m"],
                         inp["g_bn2_v"])
    w3 = inp["g_fc2_w"] * inv3[:, None]    # (96, 192)
    b3v = inp["g_fc2_b"] * inv3 + sh3
    w3T = w3.T                              # (192, 96)
    o["w3t"] = np.ascontiguousarray(
        np.concatenate([w3T[:C, :], w3T[C:, :]], axis=1)).astype(bf16)
    o["b3"] = np.ascontiguousarray(b3v[:, None], f32)

    inv4, sh4 = _fold_bn(inp["f_bn1_g"], inp["f_bn1_b"], inp["f_bn1_m"],
                         inp["f_bn1_v"])
    w4 = inp["f_fc1_w"] * inv4[:, None]    # (384, 96)
    b4v = inp["f_fc1_b"] * inv4 + sh4
    o["w4t"] = np.ascontiguousarray(w4.T).astype(bf16)   # (96, 384)
    o["b4"] = np.ascontiguousarray(b4v.reshape(3, 128).T, f32)  # (128, 3)

    inv5, sh5 = _fold_bn(inp["f_bn2_g"], inp["f_bn2_b"], inp["f_bn2_m"],
                         inp["f_bn2_v"])
    w5 = inp["f_fc2_w"] * inv5[:, None]    # (96, 384)
    b5v = inp["f_fc2_b"] * inv5 + sh5
    w5T = w5.T                              # (384, 96)
    o["w5t"] = np.ascontiguousarray(
        np.concatenate([w5T[gi * 128:(gi + 1) * 128, :] for gi in range(3)],
                       axis=1)).astype(bf16)  # (128, 288)
    o["b5"] = np.ascontiguousarray(b5v[:, None], f32)
    return o


_NC_CACHE = {}

# sigma node permutation within each 112-tile: partition p <- node sigma(p)
_SIGMA = np.array([(p % 7) * 16 + p // 7 for p in range(NT)])


def get_nc(reps: int = 1, ndev: int = NCORES):
    key = (reps, ndev)
    if key not in _NC_CACHE:
        _NC_CACHE[key] = _build_nc(reps, ndev)
    return _NC_CACHE[key]


def make_in_maps(inputs, ncores: int = NCORES):
    wts = _prep_weights({k: np.asarray(v) for k, v in inputs.items()})
    x = np.asarray(inputs["x"], np.float32).reshape(B, C, N)
    relf = np.asarray(inputs["rel_pos"], np.float32).reshape(N, NR)
    relperm = np.ascontiguousarray(
        relf.reshape(NTILES, NT, NR)[:, _SIGMA, :])
    in_maps = []
    for c in range(ncores):
        m = {"xs": np.ascontiguousarray(x[c * IPC:(c + 1) * IPC]),
             "relp": relperm}
        m.update(wts)
        in_maps.append(m)
    return in_maps


def run(inputs, reps: int = 1):
    nc = get_nc(reps)
    in_maps = make_in_maps(inputs)
    res = run_bass_kernel_spmd(nc, in_maps, list(range(NCORES)))
    out = np.concatenate([res.results[c]["out"] for c in range(NCORES)],
                         axis=0)
    return out.reshape(B, C, H, W)


def kernel(**inputs) -> np.ndarray:
    return run(inputs, reps=1)
